# revision 1
# baseline (speedup 1.0000x reference)
"""Trainium2 Bass kernel for nn_MHAEncoderFusedProj.

B=4, S=2048, E=1024, H=16, D=64, fp32. Sharding: 8 cores = 4 batch x 2
head-groups (8 heads each). No collectives: each core computes a partial
out-projection over its 512 o-features; the host adds the two partials per
batch element and transposes back.

Per-core device program (SPMD, different data per core):
  B-phase: V = x @ Wv^T (token-major) via lhsT=x^T tiles.
  A-phase: Q^T,K^T = (Wqk^T)^T-contract @ x^T, RoPE applied in
           [feature, token] layout using a signed permutation matmul for
           rotate_half plus host-tiled cos/sin.
  C-phase: per head pair, per 512-wide q chunk: scores^T[k,q] for both
           heads packed into PE row-groups, exp on ScalarE (scale=1/sqrt(D),
           no max subtraction -- scores are in [-7.2, 7.2]), PV matmul with a
           ones-column appended to V so row 64 of the o-accumulator is the
           softmax denominator. Normalization multiplies by the reciprocal
           broadcast via small SBUF-SBUF DMAs.
  D-phase: out^T_partial = (Wout-slice)^T-contract @ o^T.

All matmul operands are bitcast to float32r (FP22 truncated reads, 4x faster
than true fp32 on the PE at free-dim>=256; ~1e-4 relative error).
"""

import math

import numpy as np

P = 128
D = 64

FULL_CFG = dict(S=2048, E=1024, HG=8)


def _emit(nc, tc, io, cfg):
    import concourse.bass as bass  # noqa: F401
    import concourse.mybir as mybir

    FP32 = mybir.dt.float32
    FP32R = mybir.dt.float32r
    BF16 = mybir.dt.bfloat16
    QK_DT = BF16 if cfg.get("qk_bf16", False) else FP32R
    PV_DT = BF16 if cfg.get("pv_bf16", False) else FP32R
    EXP = mybir.ActivationFunctionType.Exp

    S, E, HG = cfg["S"], cfg["E"], cfg["HG"]
    EO = E // P              # e-tiles (contraction)
    MQK = 2 * HG * D // P    # Q+K feature tiles (rows grouped in head pairs)
    NPAIR = HG // 2
    FV = HG * D              # V features
    KT = S // P              # key token tiles
    TB = min(512, S)         # t-chunk for V projection
    NTB = S // TB
    TA = min(512, S)         # t-chunk for QK projection
    NTA = S // TA
    TR = min(512, S)         # t-chunk for rope elementwise
    NTR = S // TR
    QCH = min(512, S)        # q chunk in attention
    NQI = S // QCH
    TD = min(512, S)         # t-chunk for out projection
    NTD = S // TD
    FO = E // P              # out-proj feature tiles
    EOV = FV // P            # contraction tiles for out-proj (o features)
    scale = 1.0 / math.sqrt(D)

    def r32(ap):
        return ap.bitcast(FP32R)

    xT = io["xT"].ap()          # [E, S]
    wqkT = io["wqkT"].ap()      # [E, 2*HG*D]
    wvT = io["wvT"].ap()        # [E, HG*D]
    woutT = io["woutT"].ap()    # [HG*D, E]
    cos2T = io["cos2T"].ap()    # [P, S]
    sin2T = io["sin2T"].ap()    # [P, S]
    p2 = io["p2"].ap()          # [P, P] signed rotate-half permutation
    ones = io["ones"]           # [P, KT*HG] ones for the V ones-column
    outT = io["outT"].ap()      # [E, S]

    xT_t = xT.rearrange("(eo p) t -> p eo t", p=P)

    from concourse.tile import TileContext  # noqa: F401
    from contextlib import ExitStack

    with ExitStack() as top:
        persist = top.enter_context(tc.tile_pool(name="persist", bufs=1))
        # Global PSUM pools: "big" 2-bank slots (scores), "small" 1-bank
        # slots (everything else). Shared across phases so bank reuse does
        # not serialize cross-phase overlap. 2*2 + 4*1 = 8 banks total.
        pbig = top.enter_context(tc.tile_pool(name="pbig", bufs=2, space="PSUM"))
        psmall = top.enter_context(
            tc.tile_pool(name="psmall", bufs=4, space="PSUM")
        )
        # V with a ones column at position 64 per head: [P, KT, HG, 65]
        vsb = persist.tile([P, KT, HG, D + 1], PV_DT, tag="vsb")
        # Q^T/K^T (rope applied in place): MQK tiles of [P, S]
        qk = [persist.tile([P, S], QK_DT, tag=f"qk{m}", name=f"qk{m}") for m in range(MQK)]
        # o^T stacked by head pairs: NPAIR tiles of [P, S]
        ost = [persist.tile([P, S], FP32R, tag=f"ost{j}", name=f"ost{j}") for j in range(NPAIR)]

        # ---- Phase B: V projection (token-major via lhsT = x^T tiles) ----
        with ExitStack() as ph:
            wpool = ph.enter_context(tc.tile_pool(name="wv", bufs=1))
            xpool = ph.enter_context(tc.tile_pool(name="xb", bufs=2))

            wv = wpool.tile([P, EO, FV], FP32R)
            nc.sync.dma_start(wv, wvT.rearrange("(eo p) f -> p eo f", p=P))
            nc.sync.dma_start(vsb[:, :, :, D : D + 1], ones.ap())
            for tb in range(NTB):
                xch = xpool.tile([P, EO, TB], FP32R, tag="xb")
                nc.sync.dma_start(xch, xT_t[:, :, tb * TB : (tb + 1) * TB])
                for ts in range(TB // P):
                    ps = psmall.tile([P, FV], FP32, tag="small", name="psB")
                    for e in range(EO):
                        nc.tensor.matmul(
                            ps,
                            xch[:, e, ts * P : (ts + 1) * P],
                            wv[:, e, :],
                            start=(e == 0),
                            stop=(e == EO - 1),
                        )
                    tt = tb * (TB // P) + ts
                    nc.vector.tensor_copy(
                        vsb[:, tt, :, 0:D],
                        ps.rearrange("p (h d) -> p h d", d=D),
                    )

        # ---- Phase A: Q^T/K^T projection + RoPE ----
        with ExitStack() as ph:
            wpool = ph.enter_context(tc.tile_pool(name="wqk", bufs=2))
            xpool = ph.enter_context(tc.tile_pool(name="xa", bufs=2))

            MH = MQK // 2
            for half in range(2):
                wqk = wpool.tile([P, EO, MH * P], FP32R, tag="wqk", name="wqk")
                nc.sync.dma_start(
                    wqk,
                    wqkT[:, half * MH * P : (half + 1) * MH * P].rearrange(
                        "(eo p) f -> p eo f", p=P
                    ),
                )
                for ta in range(NTA):
                    xch = xpool.tile([P, EO, TA], FP32R, tag="xa")
                    nc.sync.dma_start(xch, xT_t[:, :, ta * TA : (ta + 1) * TA])
                    for mh in range(MH):
                        m = half * MH + mh
                        ps = psmall.tile([P, TA], FP32, tag="small", name="psA")
                        for e in range(EO):
                            nc.tensor.matmul(
                                ps,
                                wqk[:, e, mh * P : (mh + 1) * P],
                                xch[:, e, :],
                                start=(e == 0),
                                stop=(e == EO - 1),
                            )
                        nc.vector.tensor_copy(
                            qk[m][:, ta * TA : (ta + 1) * TA], ps
                        )

        # ---- Phase C: attention (rope for each pair emitted just before
        # its attention so DVE rope overlaps PE/ACT attention of the
        # previous pair) ----
        with ExitStack() as ph:
            tpool = ph.enter_context(tc.tile_pool(name="trig", bufs=1))
            tmp = ph.enter_context(tc.tile_pool(name="tmpA", bufs=2))
            ep = ph.enter_context(tc.tile_pool(name="expp", bufs=4))
            npool = ph.enter_context(tc.tile_pool(name="norm", bufs=2))

            cosb = tpool.tile([P, S], FP32)
            sinb = tpool.tile([P, S], FP32)
            p2b = tpool.tile([P, P], QK_DT)
            nc.sync.dma_start(cosb, cos2T)
            nc.sync.dma_start(sinb, sin2T)
            nc.sync.dma_start(p2b, p2)

            for hp in range(NPAIR):
                qt = qk[hp]
                ktile = qk[NPAIR + hp]
                # RoPE in place for this pair's Q and K tiles
                for m in (hp, NPAIR + hp):
                    for tr in range(NTR):
                        sl = slice(tr * TR, (tr + 1) * TR)
                        rps = psmall.tile([P, TR], FP32, tag="small", name="psR")
                        nc.tensor.matmul(
                            rps, p2b, qk[m][:, sl], start=True, stop=True
                        )
                        t1 = tmp.tile([P, TR], FP32, tag="t1")
                        nc.vector.tensor_mul(t1, qk[m][:, sl], cosb[:, sl])
                        t2 = tmp.tile([P, TR], FP32, tag="t2")
                        nc.vector.tensor_mul(t2, rps, sinb[:, sl])
                        nc.vector.tensor_add(qk[m][:, sl], t1, t2)
                for qi in range(NQI):
                    qsl = slice(qi * QCH, (qi + 1) * QCH)
                    opsAB = [
                        psmall.tile([P, QCH], FP32, tag="small", name=f"ops{hs}")
                        for hs in range(2)
                    ]
                    for kt in range(KT):
                        scps = pbig.tile([P, 2 * QCH], FP32, tag="big", name="scps")
                        ksl = slice(kt * P, (kt + 1) * P)
                        for hs in range(2):
                            b = hs * D
                            nc.tensor.matmul(
                                scps[:, hs * QCH : (hs + 1) * QCH],
                                ktile[b : b + D, ksl],
                                qt[b : b + D, qsl],
                                start=True,
                                stop=True,
                            )
                        ex = ep.tile([P, 2 * QCH], PV_DT, tag="exp")
                        nc.scalar.activation(ex, scps, EXP, scale=scale)
                        for hs in range(2):
                            nc.tensor.matmul(
                                opsAB[hs][0 : D + 1, :],
                                vsb[:, kt, 2 * hp + hs, :],
                                ex[:, hs * QCH : (hs + 1) * QCH],
                                start=(kt == 0),
                                stop=(kt == KT - 1),
                            )
                    # normalize both heads
                    for hs in range(2):
                        ops = opsAB[hs]
                        rstage = npool.tile([P, QCH], FP32, tag="rstage")
                        nc.vector.tensor_copy(
                            rstage[D : D + 1, :], ops[D : D + 1, :]
                        )
                        rs8 = npool.tile([P, QCH // P], FP32, tag="rs8")
                        nc.sync.dma_start(rs8, rstage[D : D + 1, :])
                        ri8 = npool.tile([P, QCH // P], FP32, tag="ri8")
                        nc.vector.reciprocal(ri8, rs8)
                        rifl = npool.tile([1, QCH], FP32, tag="rifl")
                        nc.sync.dma_start(rifl, ri8)
                        rbc = npool.tile([D, QCH], FP32, tag="rbc")
                        nc.gpsimd.partition_broadcast(rbc, rifl)
                        if hs == 0:
                            nc.vector.tensor_mul(
                                ost[hp][0:D, qsl], ops[0:D, :], rbc
                            )
                        else:
                            otmp = npool.tile([D, QCH], FP32R, tag="otmp")
                            nc.vector.tensor_mul(otmp, ops[0:D, :], rbc)
                            nc.sync.dma_start(ost[hp][D : 2 * D, qsl], otmp)

        # ---- Phase D: partial out-projection ----
        with ExitStack() as ph:
            wpool = ph.enter_context(tc.tile_pool(name="wo", bufs=1))
            ev = ph.enter_context(tc.tile_pool(name="evD", bufs=3))

            wo = wpool.tile([P, EOV, E], FP32R)
            nc.sync.dma_start(wo, woutT.rearrange("(eo p) f -> p eo f", p=P))
            outT_t = outT.rearrange("(fo p) t -> p fo t", p=P)
            for fo in range(FO):
                for td in range(NTD):
                    ps = psmall.tile([P, TD], FP32, tag="small", name="psD")
                    for e in range(EOV):
                        nc.tensor.matmul(
                            ps,
                            wo[:, e, fo * P : (fo + 1) * P],
                            ost[e][:, td * TD : (td + 1) * TD],
                            start=(e == 0),
                            stop=(e == EOV - 1),
                        )
                    ot = ev.tile([P, TD], FP32, tag="evD")
                    nc.vector.tensor_copy(ot, ps)
                    nc.sync.dma_start(outT_t[:, fo, td * TD : (td + 1) * TD], ot)


def _build(cfg):
    from concourse import bacc
    import concourse.mybir as mybir
    import concourse.tile as tile

    S, E, HG = cfg["S"], cfg["E"], cfg["HG"]
    FP32 = mybir.dt.float32
    FP32R = mybir.dt.float32r
    BF16 = mybir.dt.bfloat16
    QK_DT = BF16 if cfg.get("qk_bf16", False) else FP32R
    PV_DT = BF16 if cfg.get("pv_bf16", False) else FP32R
    nc = bacc.Bacc("TRN2", target_bir_lowering=False, debug=False)
    io = {
        "xT": nc.dram_tensor("xT", [E, S], FP32R, kind="ExternalInput"),
        "wqkT": nc.dram_tensor("wqkT", [E, 2 * HG * D], FP32R, kind="ExternalInput"),
        "wvT": nc.dram_tensor("wvT", [E, HG * D], FP32R, kind="ExternalInput"),
        "woutT": nc.dram_tensor("woutT", [HG * D, E], FP32R, kind="ExternalInput"),
        "cos2T": nc.dram_tensor("cos2T", [P, S], FP32, kind="ExternalInput"),
        "sin2T": nc.dram_tensor("sin2T", [P, S], FP32, kind="ExternalInput"),
        "p2": nc.dram_tensor("p2", [P, P], QK_DT, kind="ExternalInput"),
        "ones": nc.dram_tensor(
            "ones", [P, (S // P) * HG], PV_DT, kind="ExternalInput"
        ),
        "outT": nc.dram_tensor("outT", [E, S], FP32, kind="ExternalOutput"),
    }
    with tile.TileContext(nc) as tc:
        _emit(nc, tc, io, cfg)
    nc.compile()
    return nc


def _rot_matrix():
    """P2[p, m] such that (P2^T @ v) = rotate_half(v) for the 2-head
    [128]-row layout (two independent 64-blocks)."""
    p2 = np.zeros((P, P), dtype=np.float32)
    for blk in (0, 64):
        for d in range(32):
            # rot[d] = -v[d+32]  -> P2[d+32, d] = -1
            p2[blk + d + 32, blk + d] = -1.0
            # rot[d+32] = v[d]   -> P2[d, d+32] = +1
            p2[blk + d, blk + d + 32] = 1.0
    return p2


def make_core_inputs(x, cos, sin, W_qkv, W_out, cfg=FULL_CFG):
    """Host-side shard prep. Returns list of 8 in_maps."""
    S, E, HG = cfg["S"], cfg["E"], cfg["HG"]
    B = x.shape[0]
    NG = 2  # head groups
    FG = HG * D  # features per group
    cos2T = np.ascontiguousarray(np.tile(cos.T, (2, 1))).astype(np.float32)
    sin2T = np.ascontiguousarray(np.tile(sin.T, (2, 1))).astype(np.float32)
    import ml_dtypes

    qk_dt = ml_dtypes.bfloat16 if cfg.get("qk_bf16", False) else np.float32
    pv_dt = ml_dtypes.bfloat16 if cfg.get("pv_bf16", False) else np.float32
    p2 = _rot_matrix().astype(qk_dt)
    ones = np.ones((P, (S // P) * HG), dtype=pv_dt)
    xTs = [np.ascontiguousarray(x[b].T) for b in range(B)]
    in_maps = []
    for c in range(B * NG):
        b, g = c % B, c // B
        qs = slice(g * FG, (g + 1) * FG)
        ks = slice(E + g * FG, E + (g + 1) * FG)
        vs = slice(2 * E + g * FG, 2 * E + (g + 1) * FG)
        wqkT = np.ascontiguousarray(
            np.concatenate([W_qkv[qs], W_qkv[ks]], axis=0).T
        )
        wvT = np.ascontiguousarray(W_qkv[vs].T)
        woutT = np.ascontiguousarray(W_out[:, qs].T)
        in_maps.append(
            {
                "xT": xTs[b],
                "wqkT": wqkT,
                "wvT": wvT,
                "woutT": woutT,
                "cos2T": cos2T,
                "sin2T": sin2T,
                "p2": p2,
                "ones": ones,
            }
        )
    return in_maps


_NC_CACHE = {}


def _get_nc(cfg_key):
    if cfg_key not in _NC_CACHE:
        _NC_CACHE[cfg_key] = _build(
            dict(zip(("S", "E", "HG", "qk_bf16", "pv_bf16"), cfg_key))
        )
    return _NC_CACHE[cfg_key]


def kernel(x, cos, sin, W_qkv, W_out, _trace=False):
    x = np.asarray(x, dtype=np.float32)
    cos = np.asarray(cos, dtype=np.float32)
    sin = np.asarray(sin, dtype=np.float32)
    W_qkv = np.asarray(W_qkv, dtype=np.float32)
    W_out = np.asarray(W_out, dtype=np.float32)
    B, S, E = x.shape
    qk_bf16 = bool(int(__import__("os").environ.get("K_QK_BF16", "0")))
    pv_bf16 = bool(int(__import__("os").environ.get("K_PV_BF16", "0")))
    cfg = dict(S=S, E=E, HG=8, qk_bf16=qk_bf16, pv_bf16=pv_bf16)
    nc = _get_nc((S, E, 8, qk_bf16, pv_bf16))
    in_maps = make_core_inputs(x, cos, sin, W_qkv, W_out, cfg)

    from concourse.bass_utils import run_bass_kernel_spmd

    res = run_bass_kernel_spmd(
        nc, in_maps, core_ids=list(range(8)), trace=_trace
    )
    outs = [r["outT"] for r in res.results]
    out = np.empty((B, S, E), dtype=np.float32)
    for b in range(B):
        out[b] = (outs[b] + outs[b + B]).T
    kernel.last_result = res
    return out

